# revision 25
# baseline (speedup 1.0000x reference)
"""Chamfer-loss-overlap kernel for 8 Trainium2 NeuronCores.

Math (per batch element, reference semantics):
    P[i,j] = |x_i|^2 + |y_j|^2 - 2 x_i . y_j          (4096 x 4096)
    a = mean(x_mask * min_i P[i,j])    (min over i, per y-point j)
    b = mean(y_mask * min_j P[i,j])    (min over j, per x-point i)
    out = (a - b)^2
Sharding: batch dim B=8 across the 8 cores (data parallel).

Device strategy (v2):
  - The PE computes NEGATED distances -P as ONE K=13 bf16 matmul per
    128x512 PSUM bank: fp32 x/y are split hi/lo into bf16 and the norm
    terms ride along as extra contraction rows (la is negated host-side
    so all reductions become MAX, which gpsimd partition_all_reduce
    supports).
  - ScalarE (Act) casts each fp32 PSUM strip to bf16 in SBUF (0.833
    ns/col, the drain bottleneck; 2048-wide strips amortize overhead).
  - VectorE (DVE) per row-tile: one full-width tensor_tensor MAX into
    the column accumulator (2x bf16 mode) and one tensor_scalar with
    accum_out=max for the row reduction (4x mode - the big win vs the
    old fold+reduce chain).
  - Column accumulation is split into two phases (tiles 0-15 / 16-31).
    Phase A's partition-fold (gpsimd partition_all_reduce(max)) runs on
    the otherwise-idle Pool engine DURING phase B; only phase B's fold
    (split Pool / PE-transpose+DVE-reduce) is serial at the end.
  - bf16 rounding is monotone, so max(bf16(-P)) == bf16(max(-P)).
Host applies masks / means in float64 and squares the difference.
"""

import numpy as np
from ml_dtypes import bfloat16

import concourse.bacc as bacc
import concourse.bass as bass
import concourse.bass_isa as bass_isa
import concourse.mybir as mybir
from concourse import tile
from concourse import masks

B, N, D = 8, 4096, 3
NCORES = 8
NT = N // 128        # 32 row-tiles
QW = 512             # one PSUM bank of fp32 (max matmul free dim)

# --- tuning config ----------------------------------------------------
# sw: PSUM strip width (2048 = 4 banks; 2 strips double-buffer PSUM)
# pool_tail: columns of the phase-B column-fold done by gpsimd
#            partition_all_reduce (rest via PE transpose + DVE reduce).
#            Must be a multiple of 128.
# dve_copy: columns per tile cast PSUM->SBUF by DVE instead of Act
#           (rebalance knob; 0 disables). Multiple of 128.
# phases: column-accumulator phases (2 overlaps half the tail)
DEFAULT_CFG = dict(
    sw=2048,
    pool_tail=0,
    dve_copy=0,
    phases=1,
    rowacc_dt="f32",    # "f32" | "bf16" accum_out dtype
    row_mode="fold",    # "fold" (TT fold tree at 2x + short 1x reduce)
                        # | "fold2" (tile-PAIR batched fold tree: wider ops,
                        #   half the op count; pairs colmax updates too)
                        # | "ts" (tensor_scalar accum - 1x on HW)
                        # | "ts_plain" (timing-only, wrong numerics)
                        # | "pool" (gpsimd) | "split:<r>" (pool takes r cols)
    fold_to=256,        # fold tree stops at this width, then tensor_reduce
    tail_mode="par",    # "par" (gpsimd partition_all_reduce; shares the
                        # DVE SBUF port) | "tr" (PE transpose + DVE reduce)
    col_split=0,        # colmax columns updated by gpsimd instead of DVE
    ablate=None,   # None | "row" | "col" (timing experiments only)
)
# ---------------------------------------------------------------------

_CACHE = {}


def _build_nc(reps=1, **overrides):
    cfg = dict(DEFAULT_CFG, **overrides)
    dt = mybir.dt
    amax = mybir.AluOpType.max
    nc = bacc.Bacc("TRN2", target_bir_lowering=False, debug=False,
                   num_devices=NCORES)

    K = 13
    la_d = nc.dram_tensor("la", [K, N], dt.bfloat16, kind="ExternalInput")
    ra_d = nc.dram_tensor("ra", [K, N], dt.bfloat16, kind="ExternalInput")
    # negated row maxima: m[p, it] = max_j(-P[i,j]) for i = it*128+p
    acc_dt = dt.bfloat16 if cfg["rowacc_dt"] == "bf16" else dt.float32
    # split row mode: DVE handles cols [0, N-r), pool cols [N-r, N) into
    # a second accumulator
    split_r = (int(cfg["row_mode"].split(":")[1])
               if cfg["row_mode"].startswith("split") else 0)
    minsA_d = nc.dram_tensor("minsA", [128, NT], acc_dt,
                             kind="ExternalOutput")
    minsA2_d = nc.dram_tensor("minsA2", [128, NT], acc_dt,
                              kind="ExternalOutput") if split_r else None
    # per-phase column maxima (see tail below)
    nph = cfg["phases"]
    parA_d = nc.dram_tensor("parA", [nph - 1, N], dt.bfloat16,
                            kind="ExternalOutput") if nph > 1 else None
    pt = cfg["pool_tail"]
    parB_d = nc.dram_tensor("parB", [1, pt], dt.bfloat16,
                            kind="ExternalOutput") if pt else None
    ntr = (N - pt) // 128
    trB_d = nc.dram_tensor("trB", [128, ntr], dt.float32,
                           kind="ExternalOutput") if ntr else None

    with tile.TileContext(nc) as tc:
        with (
            tc.tile_pool(name="rows", bufs=1) as rows,
            tc.tile_pool(name="accs", bufs=1) as accs,
        ):
            la = rows.tile([K, N], dt.bfloat16, tag="la")
            ra = rows.tile([K, N], dt.bfloat16, tag="ra")
            # chunked loads: tile 0 only needs la[:, 0:128] and ra strip 0,
            # so the first matmuls start ~3us earlier on a one-shot run
            for c0 in range(0, N, 1024):
                nc.sync.dma_start(la[:, c0:c0 + 1024], la_d[:, c0:c0 + 1024])
                nc.sync.dma_start(ra[:, c0:c0 + 1024], ra_d[:, c0:c0 + 1024])

            ident = rows.tile([128, 128], dt.bfloat16, tag="ident")
            masks.make_identity(nc, ident[:])

            colmax = [accs.tile([128, N], dt.bfloat16, tag=f"colmax{p}",
                                name=f"colmax{p}") for p in range(nph)]
            parout = [accs.tile([128, N], dt.bfloat16, tag=f"parout{p}",
                                name=f"parout{p}") for p in range(nph - 1)]
            rowacc = accs.tile([128, NT], acc_dt, tag="rowacc")
            rowacc2 = (accs.tile([128, NT], acc_dt, tag="rowacc2",
                                 name="rowacc2") if split_r else None)

            # ablation runs skip the writer; keep outputs allocated
            if cfg["ablate"] == "row":
                nc.gpsimd.memset(rowacc[:], 0.0)
            if cfg["ablate"] == "col":
                for cm in colmax:
                    nc.gpsimd.memset(cm[:], 0.0)

            import contextlib
            rep_ctx = (tc.For_i(0, reps, 1) if reps > 1
                       else contextlib.nullcontext())
            with rep_ctx:
                _emit_main(nc, tc, la, ra, colmax, parout, rowacc, rowacc2,
                           cfg)

                # --- phase-B tail (serial part) ---
                cmB = colmax[-1]
                trB_sb = (accs.tile([128, ntr], dt.float32, tag="trB_sb",
                                    name="trB_sb") if ntr else None)
                parB_sb = (accs.tile([128, pt], dt.bfloat16, tag="parB_sb",
                                     name="parB_sb") if pt else None)
                if pt:
                    nc.gpsimd.partition_all_reduce(
                        parB_sb[:], cmB[:, 0:pt], channels=128,
                        reduce_op=bass_isa.ReduceOp.max)
                if ntr:
                    gw = 8 if ntr % 8 == 0 else 4
                    with tc.tile_pool(name="tpsum", bufs=2,
                                      space=bass.MemorySpace.PSUM) as tpsum:
                        for g in range((ntr + gw - 1) // gw):
                            nb = min(gw, ntr - g * gw)
                            pst = tpsum.tile([128, nb, 128], dt.bfloat16,
                                             tag="pst")
                            for b4 in range(nb):
                                t = g * gw + b4
                                j0 = pt + t * 128
                                nc.tensor.transpose(
                                    pst[:, b4, :],
                                    cmB[:, j0:j0 + 128],
                                    ident[:],
                                )
                            nc.vector.tensor_reduce(
                                trB_sb[:, g * gw:g * gw + nb], pst[:],
                                axis=mybir.AxisListType.X, op=amax)

            nc.sync.dma_start(minsA_d[:], rowacc[:])
            if split_r:
                nc.sync.dma_start(minsA2_d[:], rowacc2[:])
            if nph > 1:
                for p in range(nph - 1):
                    nc.sync.dma_start(parA_d[p:p + 1, :],
                                      parout[p][0:1, :])
            if pt:
                nc.sync.dma_start(parB_d[:], parB_sb[0:1, :])
            if ntr:
                nc.sync.dma_start(trB_d[:], trB_sb[:])

    nc.compile()
    return nc


def _emit_main(nc, tc, la, ra, colmax, parout, rowacc, rowacc2, cfg):
    dt = mybir.dt
    amax = mybir.AluOpType.max
    K = 13
    sw = cfg["sw"]
    nstrip = N // sw
    nph = len(colmax)
    tiles_per_phase = NT // nph
    dvc = cfg["dve_copy"]
    row_mode = cfg["row_mode"]
    split_r = int(row_mode.split(":")[1]) if row_mode.startswith("split") \
        else 0
    if row_mode == "fold2":
        _emit_main_fold2(nc, tc, la, ra, colmax, rowacc, cfg)
        return
    with (
        tc.tile_pool(name="psum", bufs=8 * QW // sw,
                     space=bass.MemorySpace.PSUM) as psum,
        tc.tile_pool(name="cpy", bufs=3) as cpy,
        tc.tile_pool(name="waste", bufs=2) as wastep,
    ):
        for it in range(NT):
            ph = it // tiles_per_phase
            first = it % tiles_per_phase == 0
            cm = colmax[ph]
            i0 = it * 128
            cp = cpy.tile([128, N], dt.bfloat16, tag="cp", name="cp")
            for h in range(nstrip):
                ps = psum.tile([128, sw], dt.float32, tag="ps", name="ps")
                for q in range(sw // QW):
                    j0 = h * sw + q * QW
                    nc.tensor.matmul(
                        ps[:, q * QW:(q + 1) * QW],
                        la[0:K, i0:i0 + 128],
                        ra[0:K, j0:j0 + QW],
                        start=True, stop=True,
                    )
                # drain: Act casts the strip (DVE takes the last dve_copy
                # columns of the tile's final strip as a rebalance assist)
                c0 = h * sw
                c1 = (h + 1) * sw
                if dvc and h == nstrip - 1:
                    nc.scalar.copy(cp[:, c0:c1 - dvc], ps[:, 0:sw - dvc])
                    nc.vector.tensor_copy(cp[:, c1 - dvc:c1],
                                          ps[:, sw - dvc:sw])
                else:
                    nc.scalar.copy(cp[:, c0:c1], ps[:])

            if cfg["ablate"] != "col":
                cs = cfg["col_split"]
                if first:
                    nc.vector.tensor_copy(cm[:, 0:N - cs], cp[:, 0:N - cs])
                    if cs:
                        nc.gpsimd.tensor_copy(cm[:, N - cs:N],
                                              cp[:, N - cs:N])
                else:
                    nc.vector.tensor_tensor(cm[:, 0:N - cs],
                                            cm[:, 0:N - cs],
                                            cp[:, 0:N - cs], amax)
                    if cs:
                        nc.gpsimd.tensor_tensor(cm[:, N - cs:N],
                                                cm[:, N - cs:N],
                                                cp[:, N - cs:N], amax)
            if cfg["ablate"] != "row":
                waste = wastep.tile([128, N], dt.bfloat16, tag="waste",
                                    name="waste")
                sl = rowacc[:, it:it + 1]
                if row_mode == "fold":
                    src = cp
                    w = N
                    while w > cfg["fold_to"]:
                        w //= 2
                        dstt = wastep.tile([128, w], dt.bfloat16,
                                           tag=f"f{w}", name=f"f{w}")
                        nc.vector.tensor_tensor(dstt[:], src[:, 0:w],
                                                src[:, w:2 * w], amax)
                        src = dstt
                    nc.vector.tensor_reduce(sl, src[:],
                                            axis=mybir.AxisListType.X,
                                            op=amax)
                elif row_mode == "ts":
                    nc.vector.tensor_scalar(
                        waste[:], cp[:], 1.0, None,
                        op0=mybir.AluOpType.mult, op1=amax,
                        accum_out=sl)
                elif row_mode == "ts_plain":  # timing probe: no accum
                    nc.vector.tensor_scalar(
                        waste[:], cp[:], 1.0, None,
                        op0=mybir.AluOpType.mult)
                    nc.vector.tensor_reduce(
                        sl, cp[:, 0:8], axis=mybir.AxisListType.X, op=amax)
                elif row_mode == "pool":
                    nc.gpsimd.tensor_scalar(
                        waste[:], cp[:], 1.0, None,
                        op0=mybir.AluOpType.mult, op1=amax,
                        accum_out=sl)
                elif split_r:
                    nc.vector.tensor_scalar(
                        waste[:, 0:N - split_r], cp[:, 0:N - split_r],
                        1.0, None,
                        op0=mybir.AluOpType.mult, op1=amax,
                        accum_out=sl)
                    nc.gpsimd.tensor_scalar(
                        waste[:, N - split_r:N], cp[:, N - split_r:N],
                        1.0, None,
                        op0=mybir.AluOpType.mult, op1=amax,
                        accum_out=rowacc2[:, it:it + 1])

            # overlapped tail: phase p's column fold runs during phase p+1
            if not first and it % tiles_per_phase == tiles_per_phase - 1 \
                    and ph < nph - 1:
                nc.gpsimd.partition_all_reduce(
                    parout[ph][:], colmax[ph][:], channels=128,
                    reduce_op=bass_isa.ReduceOp.max)


def _emit_main_fold2(nc, tc, la, ra, colmax, rowacc, cfg):
    """Tile-pair batched main loop: each cp buffer holds TWO row-tiles so
    the rowmax fold chain and final reduce run as half as many ops of
    twice the width (amortizing DVE per-op overhead), and the colmax
    update uses the pairwise-max trick (same total TT columns, but the
    phase-initial update becomes a 4x tensor_copy of the pair)."""
    dt = mybir.dt
    amax = mybir.AluOpType.max
    K = 13
    sw = cfg["sw"]
    nstrip = N // sw
    nph = len(colmax)
    tiles_per_phase = NT // nph
    assert tiles_per_phase % 2 == 0
    with (
        tc.tile_pool(name="psum", bufs=8 * QW // sw,
                     space=bass.MemorySpace.PSUM) as psum,
        tc.tile_pool(name="cpy", bufs=3) as cpy,
        tc.tile_pool(name="waste", bufs=2) as wastep,
    ):
        for pr in range(NT // 2):
            cp2 = cpy.tile([128, 2, N], dt.bfloat16, tag="cp2", name="cp2")
            for sub in range(2):
                it = pr * 2 + sub
                i0 = it * 128
                for h in range(nstrip):
                    ps = psum.tile([128, sw], dt.float32, tag="ps",
                                   name="ps")
                    for q in range(sw // QW):
                        j0 = h * sw + q * QW
                        nc.tensor.matmul(
                            ps[:, q * QW:(q + 1) * QW],
                            la[0:K, i0:i0 + 128],
                            ra[0:K, j0:j0 + QW],
                            start=True, stop=True,
                        )
                    nc.scalar.copy(cp2[:, sub, h * sw:(h + 1) * sw], ps[:])

            ph = (pr * 2) // tiles_per_phase
            cm = colmax[ph]
            pairt = wastep.tile([128, N], dt.bfloat16, tag="pair",
                                name="pairt")
            nc.vector.tensor_tensor(pairt[:], cp2[:, 0, :], cp2[:, 1, :],
                                    amax)
            if (pr * 2) % tiles_per_phase == 0:
                nc.vector.tensor_copy(cm[:], pairt[:])
            else:
                nc.vector.tensor_tensor(cm[:], cm[:], pairt[:], amax)

            src = cp2
            w = N
            while w > cfg["fold_to"]:
                w //= 2
                dstt = wastep.tile([128, 2, w], dt.bfloat16,
                                   tag=f"g{w}", name=f"g{w}")
                nc.vector.tensor_tensor(dstt[:], src[:, :, 0:w],
                                        src[:, :, w:2 * w], amax)
                src = dstt
            nc.vector.tensor_reduce(rowacc[:, pr * 2:pr * 2 + 2], src[:],
                                    axis=mybir.AxisListType.X, op=amax)


def get_nc():
    if "nc" not in _CACHE:
        _CACHE["nc"] = _build_nc()
    return _CACHE["nc"]


def _make_runner(nc):
    """Build a cached jitted SPMD callable for `nc` (one NEFF on all 8
    cores, per-core inputs sharded along axis 0)."""
    import jax
    from jax.sharding import Mesh, PartitionSpec
    from jax.experimental.shard_map import shard_map
    from concourse.bass2jax import (
        _bass_exec_p,
        install_neuronx_cc_hook,
        partition_id_tensor,
    )

    install_neuronx_cc_hook()
    partition_name = (nc.partition_id_tensor.name
                      if nc.partition_id_tensor else None)

    in_names = []
    out_names = []
    out_avals = []
    out_shapes = []
    for alloc in nc.m.functions[0].allocations:
        if not isinstance(alloc, mybir.MemoryLocationSet):
            continue
        name = alloc.memorylocations[0].name
        if alloc.kind == "ExternalInput":
            if name != partition_name:
                in_names.append(name)
        elif alloc.kind == "ExternalOutput":
            shape = tuple(alloc.tensor_shape)
            dtype = mybir.dt.np(alloc.dtype)
            out_avals.append(jax.core.ShapedArray(shape, dtype))
            out_names.append(name)
            out_shapes.append((shape, dtype))
    n_params = len(in_names)
    n_outs = len(out_names)
    all_names = list(in_names) + list(out_names)
    if partition_name is not None:
        all_names.append(partition_name)
    donate = tuple(range(n_params, n_params + n_outs))

    def _body(*args):
        operands = list(args)
        if partition_name is not None:
            operands.append(partition_id_tensor())
        outs = _bass_exec_p.bind(
            *operands,
            out_avals=tuple(out_avals),
            in_names=tuple(all_names),
            out_names=tuple(out_names),
            lowering_input_output_aliases=(),
            sim_require_finite=True,
            sim_require_nnan=True,
            nc=nc,
        )
        return tuple(outs)

    devices = jax.devices()[:NCORES]
    mesh = Mesh(np.asarray(devices), ("core",))
    sharded = jax.jit(
        shard_map(_body, mesh=mesh,
                  in_specs=(PartitionSpec("core"),) * (n_params + n_outs),
                  out_specs=(PartitionSpec("core"),) * n_outs,
                  check_rep=False),
        donate_argnums=donate,
        keep_unused=True,
    )

    def prep(in_maps):
        concat_in = [
            np.concatenate([np.asarray(m[name]) for m in in_maps], axis=0)
            for name in in_names
        ]
        return concat_in

    def exec_prepped(concat_in):
        concat_zeros = [
            np.zeros((NCORES * s[0], *s[1:]), dt) for s, dt in out_shapes
        ]
        return sharded(*concat_in, *concat_zeros)

    def unpack(out_arrs):
        return [
            {
                name: np.asarray(out_arrs[i]).reshape(
                    NCORES, *out_shapes[i][0])[c]
                for i, name in enumerate(out_names)
            }
            for c in range(NCORES)
        ]

    def run(in_maps):
        return unpack(exec_prepped(prep(in_maps)))

    run.prep = prep
    run.exec_prepped = exec_prepped
    run.unpack = unpack
    run.mesh = mesh
    return run


def get_runner():
    if "run" not in _CACHE:
        _CACHE["run"] = _make_runner(get_nc())
    return _CACHE["run"]


def _f32(v):
    return np.asarray(v, dtype=np.float32)


def _bf(v):
    return np.asarray(v, dtype=np.float32).astype(bfloat16)


def build_rows(xc, yc):
    """Build the two [13, 4096] bf16 row tensors for one batch element.

    la is NEGATED so the matmul produces -P and all on-device
    reductions are MAX.

    Contraction layout (k : L-row      * R-row):
      0-2 : -2*xh_d  * yh_d
      3-5 : -2*xl_d  * yh_d
      6-8 : -2*xh_d  * yl_d
      9   : sqx_h    * 1
      10  : sqx_l    * 1
      11  : 1        * sqy_h
      12  : 1        * sqy_l
    """
    def side(v):
        vh = _bf(v)
        vl = _bf(_f32(v) - _f32(vh))
        sq = (np.asarray(v, np.float64) ** 2).sum(-1)
        sqh = _bf(sq)
        sql = _bf(sq - np.float64(1.0) * _f32(sqh).astype(np.float64))
        m2h = _bf(-2.0 * _f32(vh))
        m2l = _bf(-2.0 * _f32(vl))
        return vh, vl, sqh, sql, m2h, m2l

    xh, xl, sqxh, sqxl, m2xh, m2xl = side(xc)
    yh, yl, sqyh, sqyl, m2yh, m2yl = side(yc)
    ones = np.ones((N,), dtype=bfloat16)

    la = np.stack([m2xh[:, 0], m2xh[:, 1], m2xh[:, 2],
                   m2xl[:, 0], m2xl[:, 1], m2xl[:, 2],
                   m2xh[:, 0], m2xh[:, 1], m2xh[:, 2],
                   sqxh, sqxl, ones, ones])
    ra = np.stack([yh[:, 0], yh[:, 1], yh[:, 2],
                   yh[:, 0], yh[:, 1], yh[:, 2],
                   yl[:, 0], yl[:, 1], yl[:, 2],
                   ones, ones, sqyh, sqyl])
    la = -la  # matmul now yields -P; device reduces with MAX
    return {
        "la": np.ascontiguousarray(la),
        "ra": np.ascontiguousarray(ra),
    }


def kernel(x, y, x_mask, y_mask):
    x = np.asarray(x)
    y = np.asarray(y)
    in_maps = [build_rows(x[c], y[c]) for c in range(B)]
    res = get_runner()(in_maps)

    cfg = DEFAULT_CFG
    pt = cfg["pool_tail"]
    sa = 0.0
    sb = 0.0
    for c in range(B):
        r = res[c]
        # minsA[p, it] = max_j(-P) for x-point it*128+p
        ra_ = np.asarray(r["minsA"], np.float64)
        if "minsA2" in r:
            ra_ = np.maximum(ra_, np.asarray(r["minsA2"], np.float64))
        minsA = -ra_.T.reshape(N)
        # column maxima: elementwise max across phase folds, then negate
        colmax = np.full(N, -np.inf)
        if "parA" in r:
            colmax = np.maximum(
                colmax,
                np.asarray(r["parA"], np.float64).max(axis=0))
        phB = np.empty(N)
        if pt:
            phB[0:pt] = np.asarray(r["parB"], np.float64)[0]
        if pt < N:
            phB[pt:] = np.asarray(r["trB"], np.float64).T.reshape(N - pt)
        colmax = np.maximum(colmax, phB)
        minsB = -colmax
        sa += (np.asarray(x_mask[c], np.float64) * minsB).sum()
        sb += (np.asarray(y_mask[c], np.float64) * minsA).sum()
    a = sa / (B * N)
    b = sb / (B * N)
    return np.asarray((a - b) ** 2, dtype=np.float32)


# revision 28
# speedup vs baseline: 1.3538x; 1.3538x over previous
"""Chamfer-loss-overlap kernel for 8 Trainium2 NeuronCores.

Math (per batch element, reference semantics):
    P[i,j] = |x_i|^2 + |y_j|^2 - 2 x_i . y_j          (4096 x 4096)
    a = mean(x_mask * min_i P[i,j])    (min over i, per y-point j)
    b = mean(y_mask * min_j P[i,j])    (min over j, per x-point i)
    out = (a - b)^2
Sharding: batch dim B=8 across the 8 cores (data parallel).

Device strategy (v2):
  - The PE computes NEGATED distances -P as ONE K=13 bf16 matmul per
    128x512 PSUM bank: fp32 x/y are split hi/lo into bf16 and the norm
    terms ride along as extra contraction rows (la is negated host-side
    so all reductions become MAX, which gpsimd partition_all_reduce
    supports).
  - ScalarE (Act) casts each fp32 PSUM strip to bf16 in SBUF (0.833
    ns/col, the drain bottleneck; 2048-wide strips amortize overhead).
  - VectorE (DVE) per row-tile: one full-width tensor_tensor MAX into
    the column accumulator (2x bf16 mode) and one tensor_scalar with
    accum_out=max for the row reduction (4x mode - the big win vs the
    old fold+reduce chain).
  - Column accumulation is split into two phases (tiles 0-15 / 16-31).
    Phase A's partition-fold (gpsimd partition_all_reduce(max)) runs on
    the otherwise-idle Pool engine DURING phase B; only phase B's fold
    (split Pool / PE-transpose+DVE-reduce) is serial at the end.
  - bf16 rounding is monotone, so max(bf16(-P)) == bf16(max(-P)).
Host applies masks / means in float64 and squares the difference.
"""

import numpy as np
from ml_dtypes import bfloat16

import concourse.bacc as bacc
import concourse.bass as bass
import concourse.bass_isa as bass_isa
import concourse.mybir as mybir
from concourse import tile
from concourse import masks

B, N, D = 8, 4096, 3
NCORES = 8
NT = N // 128        # 32 row-tiles
QW = 512             # one PSUM bank of fp32 (max matmul free dim)

# --- tuning config ----------------------------------------------------
# sw: PSUM strip width (2048 = 4 banks; 2 strips double-buffer PSUM)
# pool_tail: columns of the phase-B column-fold done by gpsimd
#            partition_all_reduce (rest via PE transpose + DVE reduce).
#            Must be a multiple of 128.
# dve_copy: columns per tile cast PSUM->SBUF by DVE instead of Act
#           (rebalance knob; 0 disables). Multiple of 128.
# phases: column-accumulator phases (2 overlaps half the tail)
DEFAULT_CFG = dict(
    sw=2048,
    pool_tail=0,
    dve_copy=0,
    phases=1,
    rowacc_dt="f32",    # "f32" | "bf16" accum_out dtype
    row_mode="fold",    # "fold" (TT fold tree at 2x + short 1x reduce)
                        # | "fold2" (tile-PAIR batched fold tree: wider ops,
                        #   half the op count; pairs colmax updates too)
                        # | "ts" (tensor_scalar accum - 1x on HW)
                        # | "ts_plain" (timing-only, wrong numerics)
                        # | "pool" (gpsimd) | "split:<r>" (pool takes r cols)
    fold_to=256,        # fold tree stops at this width, then tensor_reduce
    tail_mode="par",    # "par" (gpsimd partition_all_reduce; shares the
                        # DVE SBUF port) | "tr" (PE transpose + DVE reduce)
    col_split=0,        # colmax columns updated by gpsimd instead of DVE
    ablate=None,   # None | "row" | "col" (timing experiments only)
)
# ---------------------------------------------------------------------

_CACHE = {}


def _build_nc(reps=1, **overrides):
    cfg = dict(DEFAULT_CFG, **overrides)
    dt = mybir.dt
    amax = mybir.AluOpType.max
    nc = bacc.Bacc("TRN2", target_bir_lowering=False, debug=False,
                   num_devices=NCORES)

    K = 13
    la_d = nc.dram_tensor("la", [K, N], dt.bfloat16, kind="ExternalInput")
    ra_d = nc.dram_tensor("ra", [K, N], dt.bfloat16, kind="ExternalInput")
    # negated row maxima: m[p, it] = max_j(-P[i,j]) for i = it*128+p
    acc_dt = dt.bfloat16 if cfg["rowacc_dt"] == "bf16" else dt.float32
    # split row mode: DVE handles cols [0, N-r), pool cols [N-r, N) into
    # a second accumulator
    split_r = (int(cfg["row_mode"].split(":")[1])
               if cfg["row_mode"].startswith("split") else 0)
    minsA_d = nc.dram_tensor("minsA", [128, NT], acc_dt,
                             kind="ExternalOutput")
    minsA2_d = nc.dram_tensor("minsA2", [128, NT], acc_dt,
                              kind="ExternalOutput") if split_r else None
    # per-phase column maxima (see tail below)
    nph = cfg["phases"]
    parA_d = nc.dram_tensor("parA", [nph - 1, N], dt.bfloat16,
                            kind="ExternalOutput") if nph > 1 else None
    pt = cfg["pool_tail"]
    parB_d = nc.dram_tensor("parB", [1, pt], dt.bfloat16,
                            kind="ExternalOutput") if pt else None
    ntr = (N - pt) // 128
    trB_d = nc.dram_tensor("trB", [128, ntr], dt.float32,
                           kind="ExternalOutput") if ntr else None

    with tile.TileContext(nc) as tc:
        with (
            tc.tile_pool(name="rows", bufs=1) as rows,
            tc.tile_pool(name="accs", bufs=1) as accs,
        ):
            la = rows.tile([K, N], dt.bfloat16, tag="la")
            ra = rows.tile([K, N], dt.bfloat16, tag="ra")
            # chunked loads: tile 0 only needs la[:, 0:128] and ra strip 0,
            # so the first matmuls start ~3us earlier on a one-shot run
            for c0 in range(0, N, 1024):
                nc.sync.dma_start(la[:, c0:c0 + 1024], la_d[:, c0:c0 + 1024])
                nc.sync.dma_start(ra[:, c0:c0 + 1024], ra_d[:, c0:c0 + 1024])

            ident = rows.tile([128, 128], dt.bfloat16, tag="ident")
            masks.make_identity(nc, ident[:])

            colmax = [accs.tile([128, N], dt.bfloat16, tag=f"colmax{p}",
                                name=f"colmax{p}") for p in range(nph)]
            parout = [accs.tile([128, N], dt.bfloat16, tag=f"parout{p}",
                                name=f"parout{p}") for p in range(nph - 1)]
            rowacc = accs.tile([128, NT], acc_dt, tag="rowacc")
            rowacc2 = (accs.tile([128, NT], acc_dt, tag="rowacc2",
                                 name="rowacc2") if split_r else None)

            # ablation runs skip the writer; keep outputs allocated
            if cfg["ablate"] == "row":
                nc.gpsimd.memset(rowacc[:], 0.0)
            if cfg["ablate"] == "col":
                for cm in colmax:
                    nc.gpsimd.memset(cm[:], 0.0)

            import contextlib
            rep_ctx = (tc.For_i(0, reps, 1) if reps > 1
                       else contextlib.nullcontext())
            with rep_ctx:
                _emit_main(nc, tc, la, ra, colmax, parout, rowacc, rowacc2,
                           cfg)

                # --- phase-B tail (serial part) ---
                cmB = colmax[-1]
                trB_sb = (accs.tile([128, ntr], dt.float32, tag="trB_sb",
                                    name="trB_sb") if ntr else None)
                parB_sb = (accs.tile([128, pt], dt.bfloat16, tag="parB_sb",
                                     name="parB_sb") if pt else None)
                if pt:
                    nc.gpsimd.partition_all_reduce(
                        parB_sb[:], cmB[:, 0:pt], channels=128,
                        reduce_op=bass_isa.ReduceOp.max)
                if ntr:
                    gw = 8 if ntr % 8 == 0 else 4
                    with tc.tile_pool(name="tpsum", bufs=2,
                                      space=bass.MemorySpace.PSUM) as tpsum:
                        for g in range((ntr + gw - 1) // gw):
                            nb = min(gw, ntr - g * gw)
                            pst = tpsum.tile([128, nb, 128], dt.bfloat16,
                                             tag="pst")
                            for b4 in range(nb):
                                t = g * gw + b4
                                j0 = pt + t * 128
                                nc.tensor.transpose(
                                    pst[:, b4, :],
                                    cmB[:, j0:j0 + 128],
                                    ident[:],
                                )
                            nc.vector.tensor_reduce(
                                trB_sb[:, g * gw:g * gw + nb], pst[:],
                                axis=mybir.AxisListType.X, op=amax)

            nc.sync.dma_start(minsA_d[:], rowacc[:])
            if split_r:
                nc.sync.dma_start(minsA2_d[:], rowacc2[:])
            if nph > 1:
                for p in range(nph - 1):
                    nc.sync.dma_start(parA_d[p:p + 1, :],
                                      parout[p][0:1, :])
            if pt:
                nc.sync.dma_start(parB_d[:], parB_sb[0:1, :])
            if ntr:
                nc.sync.dma_start(trB_d[:], trB_sb[:])

    nc.compile()
    return nc


def _emit_main(nc, tc, la, ra, colmax, parout, rowacc, rowacc2, cfg):
    dt = mybir.dt
    amax = mybir.AluOpType.max
    K = 13
    sw = cfg["sw"]
    nstrip = N // sw
    nph = len(colmax)
    tiles_per_phase = NT // nph
    dvc = cfg["dve_copy"]
    row_mode = cfg["row_mode"]
    split_r = int(row_mode.split(":")[1]) if row_mode.startswith("split") \
        else 0
    if row_mode == "fold2":
        _emit_main_fold2(nc, tc, la, ra, colmax, rowacc, cfg)
        return
    with (
        tc.tile_pool(name="psum", bufs=8 * QW // sw,
                     space=bass.MemorySpace.PSUM) as psum,
        tc.tile_pool(name="cpy", bufs=4) as cpy,
        tc.tile_pool(name="waste", bufs=2) as wastep,
    ):
        for it in range(NT):
            ph = it // tiles_per_phase
            first = it % tiles_per_phase == 0
            cm = colmax[ph]
            i0 = it * 128
            cp = cpy.tile([128, N], dt.bfloat16, tag="cp", name="cp")
            for h in range(nstrip):
                ps = psum.tile([128, sw], dt.float32, tag="ps", name="ps")
                for q in range(sw // QW):
                    j0 = h * sw + q * QW
                    nc.tensor.matmul(
                        ps[:, q * QW:(q + 1) * QW],
                        la[0:K, i0:i0 + 128],
                        ra[0:K, j0:j0 + QW],
                        start=True, stop=True,
                    )
                # drain: Act casts the strip (DVE takes the last dve_copy
                # columns of the tile's final strip as a rebalance assist)
                c0 = h * sw
                c1 = (h + 1) * sw
                if dvc and h == nstrip - 1:
                    nc.scalar.copy(cp[:, c0:c1 - dvc], ps[:, 0:sw - dvc])
                    nc.vector.tensor_copy(cp[:, c1 - dvc:c1],
                                          ps[:, sw - dvc:sw])
                else:
                    nc.scalar.copy(cp[:, c0:c1], ps[:])

            def emit_colmax():
                cs = cfg["col_split"]
                if first:
                    nc.vector.tensor_copy(cm[:, 0:N - cs], cp[:, 0:N - cs])
                    if cs:
                        nc.gpsimd.tensor_copy(cm[:, N - cs:N],
                                              cp[:, N - cs:N])
                else:
                    nc.vector.tensor_tensor(cm[:, 0:N - cs],
                                            cm[:, 0:N - cs],
                                            cp[:, 0:N - cs], amax)
                    if cs:
                        nc.gpsimd.tensor_tensor(cm[:, N - cs:N],
                                                cm[:, N - cs:N],
                                                cp[:, N - cs:N], amax)

            if cfg["ablate"] != "col" and (row_mode != "fold"
                                           or cfg["ablate"] == "row"):
                emit_colmax()
            if cfg["ablate"] != "row":
                waste = wastep.tile([128, N], dt.bfloat16, tag="waste",
                                    name="waste")
                sl = rowacc[:, it:it + 1]
                if row_mode == "fold":
                    # the independent colmax update is emitted between the
                    # first two fold levels so its execution covers the
                    # dependent fold chain's inter-op pipeline bubble (and
                    # the folds cover the tile-to-tile colmax dependency)
                    src = cp
                    w = N
                    while w > cfg["fold_to"]:
                        w //= 2
                        dstt = wastep.tile([128, w], dt.bfloat16,
                                           tag=f"f{w}", name=f"f{w}")
                        nc.vector.tensor_tensor(dstt[:], src[:, 0:w],
                                                src[:, w:2 * w], amax)
                        src = dstt
                        if w == N // 2 and cfg["ablate"] != "col":
                            emit_colmax()
                    nc.vector.tensor_reduce(sl, src[:],
                                            axis=mybir.AxisListType.X,
                                            op=amax)
                elif row_mode == "ts":
                    nc.vector.tensor_scalar(
                        waste[:], cp[:], 1.0, None,
                        op0=mybir.AluOpType.mult, op1=amax,
                        accum_out=sl)
                elif row_mode == "ts_plain":  # timing probe: no accum
                    nc.vector.tensor_scalar(
                        waste[:], cp[:], 1.0, None,
                        op0=mybir.AluOpType.mult)
                    nc.vector.tensor_reduce(
                        sl, cp[:, 0:8], axis=mybir.AxisListType.X, op=amax)
                elif row_mode == "pool":
                    nc.gpsimd.tensor_scalar(
                        waste[:], cp[:], 1.0, None,
                        op0=mybir.AluOpType.mult, op1=amax,
                        accum_out=sl)
                elif split_r:
                    nc.vector.tensor_scalar(
                        waste[:, 0:N - split_r], cp[:, 0:N - split_r],
                        1.0, None,
                        op0=mybir.AluOpType.mult, op1=amax,
                        accum_out=sl)
                    nc.gpsimd.tensor_scalar(
                        waste[:, N - split_r:N], cp[:, N - split_r:N],
                        1.0, None,
                        op0=mybir.AluOpType.mult, op1=amax,
                        accum_out=rowacc2[:, it:it + 1])

            # overlapped tail: phase p's column fold runs during phase p+1
            if not first and it % tiles_per_phase == tiles_per_phase - 1 \
                    and ph < nph - 1:
                nc.gpsimd.partition_all_reduce(
                    parout[ph][:], colmax[ph][:], channels=128,
                    reduce_op=bass_isa.ReduceOp.max)


def _emit_main_fold2(nc, tc, la, ra, colmax, rowacc, cfg):
    """Tile-pair batched main loop: each cp buffer holds TWO row-tiles so
    the rowmax fold chain and final reduce run as half as many ops of
    twice the width (amortizing DVE per-op overhead), and the colmax
    update uses the pairwise-max trick (same total TT columns, but the
    phase-initial update becomes a 4x tensor_copy of the pair)."""
    dt = mybir.dt
    amax = mybir.AluOpType.max
    K = 13
    sw = cfg["sw"]
    nstrip = N // sw
    nph = len(colmax)
    tiles_per_phase = NT // nph
    assert tiles_per_phase % 2 == 0
    with (
        tc.tile_pool(name="psum", bufs=8 * QW // sw,
                     space=bass.MemorySpace.PSUM) as psum,
        tc.tile_pool(name="cpy", bufs=3) as cpy,
        tc.tile_pool(name="waste", bufs=2) as wastep,
    ):
        for pr in range(NT // 2):
            cp2 = cpy.tile([128, 2, N], dt.bfloat16, tag="cp2", name="cp2")
            for sub in range(2):
                it = pr * 2 + sub
                i0 = it * 128
                for h in range(nstrip):
                    ps = psum.tile([128, sw], dt.float32, tag="ps",
                                   name="ps")
                    for q in range(sw // QW):
                        j0 = h * sw + q * QW
                        nc.tensor.matmul(
                            ps[:, q * QW:(q + 1) * QW],
                            la[0:K, i0:i0 + 128],
                            ra[0:K, j0:j0 + QW],
                            start=True, stop=True,
                        )
                    nc.scalar.copy(cp2[:, sub, h * sw:(h + 1) * sw], ps[:])

            ph = (pr * 2) // tiles_per_phase
            cm = colmax[ph]
            pairt = wastep.tile([128, N], dt.bfloat16, tag="pair",
                                name="pairt")
            nc.vector.tensor_tensor(pairt[:], cp2[:, 0, :], cp2[:, 1, :],
                                    amax)
            if (pr * 2) % tiles_per_phase == 0:
                nc.vector.tensor_copy(cm[:], pairt[:])
            else:
                nc.vector.tensor_tensor(cm[:], cm[:], pairt[:], amax)

            src = cp2
            w = N
            while w > cfg["fold_to"]:
                w //= 2
                dstt = wastep.tile([128, 2, w], dt.bfloat16,
                                   tag=f"g{w}", name=f"g{w}")
                nc.vector.tensor_tensor(dstt[:], src[:, :, 0:w],
                                        src[:, :, w:2 * w], amax)
                src = dstt
            nc.vector.tensor_reduce(rowacc[:, pr * 2:pr * 2 + 2], src[:],
                                    axis=mybir.AxisListType.X, op=amax)


def get_nc():
    if "nc" not in _CACHE:
        _CACHE["nc"] = _build_nc()
    return _CACHE["nc"]


def _make_runner(nc):
    """Build a cached jitted SPMD callable for `nc` (one NEFF on all 8
    cores, per-core inputs sharded along axis 0)."""
    import jax
    from jax.sharding import Mesh, PartitionSpec
    from jax.experimental.shard_map import shard_map
    from concourse.bass2jax import (
        _bass_exec_p,
        install_neuronx_cc_hook,
        partition_id_tensor,
    )

    install_neuronx_cc_hook()
    partition_name = (nc.partition_id_tensor.name
                      if nc.partition_id_tensor else None)

    in_names = []
    out_names = []
    out_avals = []
    out_shapes = []
    for alloc in nc.m.functions[0].allocations:
        if not isinstance(alloc, mybir.MemoryLocationSet):
            continue
        name = alloc.memorylocations[0].name
        if alloc.kind == "ExternalInput":
            if name != partition_name:
                in_names.append(name)
        elif alloc.kind == "ExternalOutput":
            shape = tuple(alloc.tensor_shape)
            dtype = mybir.dt.np(alloc.dtype)
            out_avals.append(jax.core.ShapedArray(shape, dtype))
            out_names.append(name)
            out_shapes.append((shape, dtype))
    n_params = len(in_names)
    n_outs = len(out_names)
    all_names = list(in_names) + list(out_names)
    if partition_name is not None:
        all_names.append(partition_name)
    donate = tuple(range(n_params, n_params + n_outs))

    def _body(*args):
        operands = list(args)
        if partition_name is not None:
            operands.append(partition_id_tensor())
        outs = _bass_exec_p.bind(
            *operands,
            out_avals=tuple(out_avals),
            in_names=tuple(all_names),
            out_names=tuple(out_names),
            lowering_input_output_aliases=(),
            sim_require_finite=True,
            sim_require_nnan=True,
            nc=nc,
        )
        return tuple(outs)

    devices = jax.devices()[:NCORES]
    mesh = Mesh(np.asarray(devices), ("core",))
    sharded = jax.jit(
        shard_map(_body, mesh=mesh,
                  in_specs=(PartitionSpec("core"),) * (n_params + n_outs),
                  out_specs=(PartitionSpec("core"),) * n_outs,
                  check_rep=False),
        donate_argnums=donate,
        keep_unused=True,
    )

    def prep(in_maps):
        concat_in = [
            np.concatenate([np.asarray(m[name]) for m in in_maps], axis=0)
            for name in in_names
        ]
        return concat_in

    def exec_prepped(concat_in):
        concat_zeros = [
            np.zeros((NCORES * s[0], *s[1:]), dt) for s, dt in out_shapes
        ]
        return sharded(*concat_in, *concat_zeros)

    def unpack(out_arrs):
        return [
            {
                name: np.asarray(out_arrs[i]).reshape(
                    NCORES, *out_shapes[i][0])[c]
                for i, name in enumerate(out_names)
            }
            for c in range(NCORES)
        ]

    def run(in_maps):
        return unpack(exec_prepped(prep(in_maps)))

    run.prep = prep
    run.exec_prepped = exec_prepped
    run.unpack = unpack
    run.mesh = mesh
    return run


def get_runner():
    if "run" not in _CACHE:
        _CACHE["run"] = _make_runner(get_nc())
    return _CACHE["run"]


def _f32(v):
    return np.asarray(v, dtype=np.float32)


def _bf(v):
    return np.asarray(v, dtype=np.float32).astype(bfloat16)


def build_rows(xc, yc):
    """Build the two [13, 4096] bf16 row tensors for one batch element.

    la is NEGATED so the matmul produces -P and all on-device
    reductions are MAX.

    Contraction layout (k : L-row      * R-row):
      0-2 : -2*xh_d  * yh_d
      3-5 : -2*xl_d  * yh_d
      6-8 : -2*xh_d  * yl_d
      9   : sqx_h    * 1
      10  : sqx_l    * 1
      11  : 1        * sqy_h
      12  : 1        * sqy_l
    """
    def side(v):
        vh = _bf(v)
        vl = _bf(_f32(v) - _f32(vh))
        sq = (np.asarray(v, np.float64) ** 2).sum(-1)
        sqh = _bf(sq)
        sql = _bf(sq - np.float64(1.0) * _f32(sqh).astype(np.float64))
        m2h = _bf(-2.0 * _f32(vh))
        m2l = _bf(-2.0 * _f32(vl))
        return vh, vl, sqh, sql, m2h, m2l

    xh, xl, sqxh, sqxl, m2xh, m2xl = side(xc)
    yh, yl, sqyh, sqyl, m2yh, m2yl = side(yc)
    ones = np.ones((N,), dtype=bfloat16)

    la = np.stack([m2xh[:, 0], m2xh[:, 1], m2xh[:, 2],
                   m2xl[:, 0], m2xl[:, 1], m2xl[:, 2],
                   m2xh[:, 0], m2xh[:, 1], m2xh[:, 2],
                   sqxh, sqxl, ones, ones])
    ra = np.stack([yh[:, 0], yh[:, 1], yh[:, 2],
                   yh[:, 0], yh[:, 1], yh[:, 2],
                   yl[:, 0], yl[:, 1], yl[:, 2],
                   ones, ones, sqyh, sqyl])
    la = -la  # matmul now yields -P; device reduces with MAX
    return {
        "la": np.ascontiguousarray(la),
        "ra": np.ascontiguousarray(ra),
    }


def kernel(x, y, x_mask, y_mask):
    x = np.asarray(x)
    y = np.asarray(y)
    in_maps = [build_rows(x[c], y[c]) for c in range(B)]
    res = get_runner()(in_maps)

    cfg = DEFAULT_CFG
    pt = cfg["pool_tail"]
    sa = 0.0
    sb = 0.0
    for c in range(B):
        r = res[c]
        # minsA[p, it] = max_j(-P) for x-point it*128+p
        ra_ = np.asarray(r["minsA"], np.float64)
        if "minsA2" in r:
            ra_ = np.maximum(ra_, np.asarray(r["minsA2"], np.float64))
        minsA = -ra_.T.reshape(N)
        # column maxima: elementwise max across phase folds, then negate
        colmax = np.full(N, -np.inf)
        if "parA" in r:
            colmax = np.maximum(
                colmax,
                np.asarray(r["parA"], np.float64).max(axis=0))
        phB = np.empty(N)
        if pt:
            phB[0:pt] = np.asarray(r["parB"], np.float64)[0]
        if pt < N:
            phB[pt:] = np.asarray(r["trB"], np.float64).T.reshape(N - pt)
        colmax = np.maximum(colmax, phB)
        minsB = -colmax
        sa += (np.asarray(x_mask[c], np.float64) * minsB).sum()
        sb += (np.asarray(y_mask[c], np.float64) * minsA).sum()
    a = sa / (B * N)
    b = sb / (B * N)
    return np.asarray((a - b) ** 2, dtype=np.float32)


# revision 29
# speedup vs baseline: 1.3726x; 1.0139x over previous
"""Chamfer-loss-overlap kernel for 8 Trainium2 NeuronCores.

Math (per batch element, reference semantics):
    P[i,j] = |x_i|^2 + |y_j|^2 - 2 x_i . y_j          (4096 x 4096)
    a = mean(x_mask * min_i P[i,j])    (min over i, per y-point j)
    b = mean(y_mask * min_j P[i,j])    (min over j, per x-point i)
    out = (a - b)^2
Sharding: batch dim B=8 across the 8 cores (data parallel).

Device strategy (v3, HW-measured rates in parens):
  - The PE computes NEGATED distances -P as ONE K=13 bf16 matmul per
    128x512 PSUM bank: fp32 x/y are split hi/lo into bf16 and the norm
    terms ride along as extra contraction rows. la is negated host-side
    so every on-device reduction is a MAX.
  - ScalarE casts each fp32 PSUM strip to bf16 in SBUF (0.94 ns/col;
    2048-wide strips; fully hidden under DVE).
  - VectorE (DVE, the bottleneck at ~100% occupancy) per row-tile: a
    tensor_tensor max fold tree 4096->256 (2x mode, 0.574 ns/col) with
    the full-width column-accumulator TT interleaved into the chain,
    then a short 1x tensor_reduce. (tensor_scalar+accum_out measures 1x
    on HW despite the cost model's 4x; gpsimd offload loses more to
    DVE SBUF-port contention than it saves; see project memory.)
  - Column-min tail: PE transposes + 4 wide DVE reduces after the loop,
    when PSUM is free.
  - bf16 rounding is monotone, so max(bf16(-P)) == bf16(max(-P)).
Host applies masks / means in float64 and squares the difference.
"""

import numpy as np
from ml_dtypes import bfloat16

import concourse.bacc as bacc
import concourse.bass as bass
import concourse.bass_isa as bass_isa
import concourse.mybir as mybir
from concourse import tile
from concourse import masks

B, N, D = 8, 4096, 3
NCORES = 8
NT = N // 128        # 32 row-tiles
QW = 512             # one PSUM bank of fp32 (max matmul free dim)

# --- tuning config ----------------------------------------------------
# sw: PSUM strip width (2048 = 4 banks; 2 strips double-buffer PSUM)
# pool_tail: columns of the phase-B column-fold done by gpsimd
#            partition_all_reduce (rest via PE transpose + DVE reduce).
#            Must be a multiple of 128.
# dve_copy: columns per tile cast PSUM->SBUF by DVE instead of Act
#           (rebalance knob; 0 disables). Multiple of 128.
# phases: column-accumulator phases (2 overlaps half the tail)
DEFAULT_CFG = dict(
    sw=2048,
    pool_tail=0,
    dve_copy=0,
    phases=1,
    rowacc_dt="f32",    # "f32" | "bf16" accum_out dtype
    row_mode="fold",    # "fold" (TT fold tree at 2x + short 1x reduce)
                        # | "fold2" (tile-PAIR batched fold tree: wider ops,
                        #   half the op count; pairs colmax updates too)
                        # | "ts" (tensor_scalar accum - 1x on HW)
                        # | "ts_plain" (timing-only, wrong numerics)
                        # | "pool" (gpsimd) | "split:<r>" (pool takes r cols)
    fold_to=256,        # fold tree stops at this width, then tensor_reduce
    tail_mode="par",    # "par" (gpsimd partition_all_reduce; shares the
                        # DVE SBUF port) | "tr" (PE transpose + DVE reduce)
    col_split=0,        # colmax columns updated by gpsimd instead of DVE
    ablate=None,   # None | "row" | "col" (timing experiments only)
)
# ---------------------------------------------------------------------

_CACHE = {}


def _build_nc(reps=1, **overrides):
    cfg = dict(DEFAULT_CFG, **overrides)
    dt = mybir.dt
    amax = mybir.AluOpType.max
    nc = bacc.Bacc("TRN2", target_bir_lowering=False, debug=False,
                   num_devices=NCORES)

    K = 13
    la_d = nc.dram_tensor("la", [K, N], dt.bfloat16, kind="ExternalInput")
    ra_d = nc.dram_tensor("ra", [K, N], dt.bfloat16, kind="ExternalInput")
    # negated row maxima: m[p, it] = max_j(-P[i,j]) for i = it*128+p
    acc_dt = dt.bfloat16 if cfg["rowacc_dt"] == "bf16" else dt.float32
    # split row mode: DVE handles cols [0, N-r), pool cols [N-r, N) into
    # a second accumulator
    split_r = (int(cfg["row_mode"].split(":")[1])
               if cfg["row_mode"].startswith("split") else 0)
    minsA_d = nc.dram_tensor("minsA", [128, NT], acc_dt,
                             kind="ExternalOutput")
    minsA2_d = nc.dram_tensor("minsA2", [128, NT], acc_dt,
                              kind="ExternalOutput") if split_r else None
    # per-phase column maxima (see tail below)
    nph = cfg["phases"]
    parA_d = nc.dram_tensor("parA", [nph - 1, N], dt.bfloat16,
                            kind="ExternalOutput") if nph > 1 else None
    pt = cfg["pool_tail"]
    parB_d = nc.dram_tensor("parB", [1, pt], dt.bfloat16,
                            kind="ExternalOutput") if pt else None
    ntr = (N - pt) // 128
    trB_d = nc.dram_tensor("trB", [128, ntr], dt.float32,
                           kind="ExternalOutput") if ntr else None

    with tile.TileContext(nc) as tc:
        with (
            tc.tile_pool(name="rows", bufs=1) as rows,
            tc.tile_pool(name="accs", bufs=1) as accs,
        ):
            la = rows.tile([K, N], dt.bfloat16, tag="la")
            ra = rows.tile([K, N], dt.bfloat16, tag="ra")
            # chunked loads: tile 0 only needs la[:, 0:128] and ra strip 0,
            # so the first matmuls start ~3us earlier on a one-shot run
            for c0 in range(0, N, 1024):
                nc.sync.dma_start(la[:, c0:c0 + 1024], la_d[:, c0:c0 + 1024])
                nc.sync.dma_start(ra[:, c0:c0 + 1024], ra_d[:, c0:c0 + 1024])

            ident = rows.tile([128, 128], dt.bfloat16, tag="ident")
            masks.make_identity(nc, ident[:])

            colmax = [accs.tile([128, N], dt.bfloat16, tag=f"colmax{p}",
                                name=f"colmax{p}") for p in range(nph)]
            parout = [accs.tile([128, N], dt.bfloat16, tag=f"parout{p}",
                                name=f"parout{p}") for p in range(nph - 1)]
            rowacc = accs.tile([128, NT], acc_dt, tag="rowacc")
            rowacc2 = (accs.tile([128, NT], acc_dt, tag="rowacc2",
                                 name="rowacc2") if split_r else None)

            # ablation runs skip the writer; keep outputs allocated
            if cfg["ablate"] == "row":
                nc.gpsimd.memset(rowacc[:], 0.0)
            if cfg["ablate"] == "col":
                for cm in colmax:
                    nc.gpsimd.memset(cm[:], 0.0)

            import contextlib
            rep_ctx = (tc.For_i(0, reps, 1) if reps > 1
                       else contextlib.nullcontext())
            with rep_ctx:
                _emit_main(nc, tc, la, ra, colmax, parout, rowacc, rowacc2,
                           cfg)

                # --- phase-B tail (serial part) ---
                cmB = colmax[-1]
                trB_sb = (accs.tile([128, ntr], dt.float32, tag="trB_sb",
                                    name="trB_sb") if ntr else None)
                parB_sb = (accs.tile([128, pt], dt.bfloat16, tag="parB_sb",
                                     name="parB_sb") if pt else None)
                if pt:
                    nc.gpsimd.partition_all_reduce(
                        parB_sb[:], cmB[:, 0:pt], channels=128,
                        reduce_op=bass_isa.ReduceOp.max)
                if ntr:
                    gw = 8 if ntr % 8 == 0 else 4
                    with tc.tile_pool(name="tpsum", bufs=2,
                                      space=bass.MemorySpace.PSUM) as tpsum:
                        for g in range((ntr + gw - 1) // gw):
                            nb = min(gw, ntr - g * gw)
                            pst = tpsum.tile([128, nb, 128], dt.bfloat16,
                                             tag="pst")
                            for b4 in range(nb):
                                t = g * gw + b4
                                j0 = pt + t * 128
                                nc.tensor.transpose(
                                    pst[:, b4, :],
                                    cmB[:, j0:j0 + 128],
                                    ident[:],
                                )
                            nc.vector.tensor_reduce(
                                trB_sb[:, g * gw:g * gw + nb], pst[:],
                                axis=mybir.AxisListType.X, op=amax)

            nc.sync.dma_start(minsA_d[:], rowacc[:])
            if split_r:
                nc.sync.dma_start(minsA2_d[:], rowacc2[:])
            if nph > 1:
                for p in range(nph - 1):
                    nc.sync.dma_start(parA_d[p:p + 1, :],
                                      parout[p][0:1, :])
            if pt:
                nc.sync.dma_start(parB_d[:], parB_sb[0:1, :])
            if ntr:
                nc.sync.dma_start(trB_d[:], trB_sb[:])

    nc.compile()
    return nc


def _emit_main(nc, tc, la, ra, colmax, parout, rowacc, rowacc2, cfg):
    dt = mybir.dt
    amax = mybir.AluOpType.max
    K = 13
    sw = cfg["sw"]
    nstrip = N // sw
    nph = len(colmax)
    tiles_per_phase = NT // nph
    dvc = cfg["dve_copy"]
    row_mode = cfg["row_mode"]
    split_r = int(row_mode.split(":")[1]) if row_mode.startswith("split") \
        else 0
    if row_mode == "fold2":
        _emit_main_fold2(nc, tc, la, ra, colmax, rowacc, cfg)
        return
    with (
        tc.tile_pool(name="psum", bufs=8 * QW // sw,
                     space=bass.MemorySpace.PSUM) as psum,
        tc.tile_pool(name="cpy", bufs=4) as cpy,
        tc.tile_pool(name="waste", bufs=2) as wastep,
    ):
        for it in range(NT):
            ph = it // tiles_per_phase
            first = it % tiles_per_phase == 0
            cm = colmax[ph]
            i0 = it * 128
            cp = cpy.tile([128, N], dt.bfloat16, tag="cp", name="cp")
            for h in range(nstrip):
                ps = psum.tile([128, sw], dt.float32, tag="ps", name="ps")
                for q in range(sw // QW):
                    j0 = h * sw + q * QW
                    nc.tensor.matmul(
                        ps[:, q * QW:(q + 1) * QW],
                        la[0:K, i0:i0 + 128],
                        ra[0:K, j0:j0 + QW],
                        start=True, stop=True,
                    )
                # drain: Act casts the strip (DVE takes the last dve_copy
                # columns of the tile's final strip as a rebalance assist)
                c0 = h * sw
                c1 = (h + 1) * sw
                if dvc and h == nstrip - 1:
                    nc.scalar.copy(cp[:, c0:c1 - dvc], ps[:, 0:sw - dvc])
                    nc.vector.tensor_copy(cp[:, c1 - dvc:c1],
                                          ps[:, sw - dvc:sw])
                else:
                    nc.scalar.copy(cp[:, c0:c1], ps[:])

            def emit_colmax():
                cs = cfg["col_split"]
                if first:
                    nc.vector.tensor_copy(cm[:, 0:N - cs], cp[:, 0:N - cs])
                    if cs:
                        nc.gpsimd.tensor_copy(cm[:, N - cs:N],
                                              cp[:, N - cs:N])
                else:
                    nc.vector.tensor_tensor(cm[:, 0:N - cs],
                                            cm[:, 0:N - cs],
                                            cp[:, 0:N - cs], amax)
                    if cs:
                        nc.gpsimd.tensor_tensor(cm[:, N - cs:N],
                                                cm[:, N - cs:N],
                                                cp[:, N - cs:N], amax)

            if cfg["ablate"] != "col" and (row_mode != "fold"
                                           or cfg["ablate"] == "row"):
                emit_colmax()
            if cfg["ablate"] != "row":
                waste = wastep.tile([128, N], dt.bfloat16, tag="waste",
                                    name="waste")
                sl = rowacc[:, it:it + 1]
                if row_mode == "fold":
                    # the independent colmax update is emitted between the
                    # first two fold levels so its execution covers the
                    # dependent fold chain's inter-op pipeline bubble (and
                    # the folds cover the tile-to-tile colmax dependency)
                    src = cp
                    w = N
                    while w > cfg["fold_to"]:
                        w //= 2
                        dstt = wastep.tile([128, w], dt.bfloat16,
                                           tag=f"f{w}", name=f"f{w}")
                        nc.vector.tensor_tensor(dstt[:], src[:, 0:w],
                                                src[:, w:2 * w], amax)
                        src = dstt
                        if w == N // 2 and cfg["ablate"] != "col":
                            emit_colmax()
                    nc.vector.tensor_reduce(sl, src[:],
                                            axis=mybir.AxisListType.X,
                                            op=amax)
                elif row_mode == "ts":
                    nc.vector.tensor_scalar(
                        waste[:], cp[:], 1.0, None,
                        op0=mybir.AluOpType.mult, op1=amax,
                        accum_out=sl)
                elif row_mode == "ts_plain":  # timing probe: no accum
                    nc.vector.tensor_scalar(
                        waste[:], cp[:], 1.0, None,
                        op0=mybir.AluOpType.mult)
                    nc.vector.tensor_reduce(
                        sl, cp[:, 0:8], axis=mybir.AxisListType.X, op=amax)
                elif row_mode == "pool":
                    nc.gpsimd.tensor_scalar(
                        waste[:], cp[:], 1.0, None,
                        op0=mybir.AluOpType.mult, op1=amax,
                        accum_out=sl)
                elif split_r:
                    nc.vector.tensor_scalar(
                        waste[:, 0:N - split_r], cp[:, 0:N - split_r],
                        1.0, None,
                        op0=mybir.AluOpType.mult, op1=amax,
                        accum_out=sl)
                    nc.gpsimd.tensor_scalar(
                        waste[:, N - split_r:N], cp[:, N - split_r:N],
                        1.0, None,
                        op0=mybir.AluOpType.mult, op1=amax,
                        accum_out=rowacc2[:, it:it + 1])

            # overlapped tail: phase p's column fold runs during phase p+1
            if not first and it % tiles_per_phase == tiles_per_phase - 1 \
                    and ph < nph - 1:
                nc.gpsimd.partition_all_reduce(
                    parout[ph][:], colmax[ph][:], channels=128,
                    reduce_op=bass_isa.ReduceOp.max)


def _emit_main_fold2(nc, tc, la, ra, colmax, rowacc, cfg):
    """Tile-pair batched main loop: each cp buffer holds TWO row-tiles so
    the rowmax fold chain and final reduce run as half as many ops of
    twice the width (amortizing DVE per-op overhead), and the colmax
    update uses the pairwise-max trick (same total TT columns, but the
    phase-initial update becomes a 4x tensor_copy of the pair)."""
    dt = mybir.dt
    amax = mybir.AluOpType.max
    K = 13
    sw = cfg["sw"]
    nstrip = N // sw
    nph = len(colmax)
    tiles_per_phase = NT // nph
    assert tiles_per_phase % 2 == 0
    with (
        tc.tile_pool(name="psum", bufs=8 * QW // sw,
                     space=bass.MemorySpace.PSUM) as psum,
        tc.tile_pool(name="cpy", bufs=3) as cpy,
        tc.tile_pool(name="waste", bufs=2) as wastep,
    ):
        for pr in range(NT // 2):
            cp2 = cpy.tile([128, 2, N], dt.bfloat16, tag="cp2", name="cp2")
            for sub in range(2):
                it = pr * 2 + sub
                i0 = it * 128
                for h in range(nstrip):
                    ps = psum.tile([128, sw], dt.float32, tag="ps",
                                   name="ps")
                    for q in range(sw // QW):
                        j0 = h * sw + q * QW
                        nc.tensor.matmul(
                            ps[:, q * QW:(q + 1) * QW],
                            la[0:K, i0:i0 + 128],
                            ra[0:K, j0:j0 + QW],
                            start=True, stop=True,
                        )
                    nc.scalar.copy(cp2[:, sub, h * sw:(h + 1) * sw], ps[:])

            ph = (pr * 2) // tiles_per_phase
            cm = colmax[ph]
            pairt = wastep.tile([128, N], dt.bfloat16, tag="pair",
                                name="pairt")
            nc.vector.tensor_tensor(pairt[:], cp2[:, 0, :], cp2[:, 1, :],
                                    amax)
            if (pr * 2) % tiles_per_phase == 0:
                nc.vector.tensor_copy(cm[:], pairt[:])
            else:
                nc.vector.tensor_tensor(cm[:], cm[:], pairt[:], amax)

            src = cp2
            w = N
            while w > cfg["fold_to"]:
                w //= 2
                dstt = wastep.tile([128, 2, w], dt.bfloat16,
                                   tag=f"g{w}", name=f"g{w}")
                nc.vector.tensor_tensor(dstt[:], src[:, :, 0:w],
                                        src[:, :, w:2 * w], amax)
                src = dstt
            nc.vector.tensor_reduce(rowacc[:, pr * 2:pr * 2 + 2], src[:],
                                    axis=mybir.AxisListType.X, op=amax)


def get_nc():
    if "nc" not in _CACHE:
        _CACHE["nc"] = _build_nc()
    return _CACHE["nc"]


def _make_runner(nc):
    """Build a cached jitted SPMD callable for `nc` (one NEFF on all 8
    cores, per-core inputs sharded along axis 0)."""
    import jax
    from jax.sharding import Mesh, PartitionSpec
    from jax.experimental.shard_map import shard_map
    from concourse.bass2jax import (
        _bass_exec_p,
        install_neuronx_cc_hook,
        partition_id_tensor,
    )

    install_neuronx_cc_hook()
    partition_name = (nc.partition_id_tensor.name
                      if nc.partition_id_tensor else None)

    in_names = []
    out_names = []
    out_avals = []
    out_shapes = []
    for alloc in nc.m.functions[0].allocations:
        if not isinstance(alloc, mybir.MemoryLocationSet):
            continue
        name = alloc.memorylocations[0].name
        if alloc.kind == "ExternalInput":
            if name != partition_name:
                in_names.append(name)
        elif alloc.kind == "ExternalOutput":
            shape = tuple(alloc.tensor_shape)
            dtype = mybir.dt.np(alloc.dtype)
            out_avals.append(jax.core.ShapedArray(shape, dtype))
            out_names.append(name)
            out_shapes.append((shape, dtype))
    n_params = len(in_names)
    n_outs = len(out_names)
    all_names = list(in_names) + list(out_names)
    if partition_name is not None:
        all_names.append(partition_name)
    donate = tuple(range(n_params, n_params + n_outs))

    def _body(*args):
        operands = list(args)
        if partition_name is not None:
            operands.append(partition_id_tensor())
        outs = _bass_exec_p.bind(
            *operands,
            out_avals=tuple(out_avals),
            in_names=tuple(all_names),
            out_names=tuple(out_names),
            lowering_input_output_aliases=(),
            sim_require_finite=True,
            sim_require_nnan=True,
            nc=nc,
        )
        return tuple(outs)

    devices = jax.devices()[:NCORES]
    mesh = Mesh(np.asarray(devices), ("core",))
    sharded = jax.jit(
        shard_map(_body, mesh=mesh,
                  in_specs=(PartitionSpec("core"),) * (n_params + n_outs),
                  out_specs=(PartitionSpec("core"),) * n_outs,
                  check_rep=False),
        donate_argnums=donate,
        keep_unused=True,
    )

    def prep(in_maps):
        concat_in = [
            np.concatenate([np.asarray(m[name]) for m in in_maps], axis=0)
            for name in in_names
        ]
        return concat_in

    def exec_prepped(concat_in):
        concat_zeros = [
            np.zeros((NCORES * s[0], *s[1:]), dt) for s, dt in out_shapes
        ]
        return sharded(*concat_in, *concat_zeros)

    def unpack(out_arrs):
        return [
            {
                name: np.asarray(out_arrs[i]).reshape(
                    NCORES, *out_shapes[i][0])[c]
                for i, name in enumerate(out_names)
            }
            for c in range(NCORES)
        ]

    def run(in_maps):
        return unpack(exec_prepped(prep(in_maps)))

    run.prep = prep
    run.exec_prepped = exec_prepped
    run.unpack = unpack
    run.mesh = mesh
    return run


def get_runner():
    if "run" not in _CACHE:
        _CACHE["run"] = _make_runner(get_nc())
    return _CACHE["run"]


def _f32(v):
    return np.asarray(v, dtype=np.float32)


def _bf(v):
    return np.asarray(v, dtype=np.float32).astype(bfloat16)


def build_rows(xc, yc):
    """Build the two [13, 4096] bf16 row tensors for one batch element.

    la is NEGATED so the matmul produces -P and all on-device
    reductions are MAX.

    Contraction layout (k : L-row      * R-row):
      0-2 : -2*xh_d  * yh_d
      3-5 : -2*xl_d  * yh_d
      6-8 : -2*xh_d  * yl_d
      9   : sqx_h    * 1
      10  : sqx_l    * 1
      11  : 1        * sqy_h
      12  : 1        * sqy_l
    """
    def side(v):
        vh = _bf(v)
        vl = _bf(_f32(v) - _f32(vh))
        sq = (np.asarray(v, np.float64) ** 2).sum(-1)
        sqh = _bf(sq)
        sql = _bf(sq - np.float64(1.0) * _f32(sqh).astype(np.float64))
        m2h = _bf(-2.0 * _f32(vh))
        m2l = _bf(-2.0 * _f32(vl))
        return vh, vl, sqh, sql, m2h, m2l

    xh, xl, sqxh, sqxl, m2xh, m2xl = side(xc)
    yh, yl, sqyh, sqyl, m2yh, m2yl = side(yc)
    ones = np.ones((N,), dtype=bfloat16)

    la = np.stack([m2xh[:, 0], m2xh[:, 1], m2xh[:, 2],
                   m2xl[:, 0], m2xl[:, 1], m2xl[:, 2],
                   m2xh[:, 0], m2xh[:, 1], m2xh[:, 2],
                   sqxh, sqxl, ones, ones])
    ra = np.stack([yh[:, 0], yh[:, 1], yh[:, 2],
                   yh[:, 0], yh[:, 1], yh[:, 2],
                   yl[:, 0], yl[:, 1], yl[:, 2],
                   ones, ones, sqyh, sqyl])
    la = -la  # matmul now yields -P; device reduces with MAX
    return {
        "la": np.ascontiguousarray(la),
        "ra": np.ascontiguousarray(ra),
    }


def kernel(x, y, x_mask, y_mask):
    x = np.asarray(x)
    y = np.asarray(y)
    in_maps = [build_rows(x[c], y[c]) for c in range(B)]
    res = get_runner()(in_maps)

    cfg = DEFAULT_CFG
    pt = cfg["pool_tail"]
    sa = 0.0
    sb = 0.0
    for c in range(B):
        r = res[c]
        # minsA[p, it] = max_j(-P) for x-point it*128+p
        ra_ = np.asarray(r["minsA"], np.float64)
        if "minsA2" in r:
            ra_ = np.maximum(ra_, np.asarray(r["minsA2"], np.float64))
        minsA = -ra_.T.reshape(N)
        # column maxima: elementwise max across phase folds, then negate
        colmax = np.full(N, -np.inf)
        if "parA" in r:
            colmax = np.maximum(
                colmax,
                np.asarray(r["parA"], np.float64).max(axis=0))
        phB = np.empty(N)
        if pt:
            phB[0:pt] = np.asarray(r["parB"], np.float64)[0]
        if pt < N:
            phB[pt:] = np.asarray(r["trB"], np.float64).T.reshape(N - pt)
        colmax = np.maximum(colmax, phB)
        minsB = -colmax
        sa += (np.asarray(x_mask[c], np.float64) * minsB).sum()
        sb += (np.asarray(y_mask[c], np.float64) * minsA).sum()
    a = sa / (B * N)
    b = sb / (B * N)
    return np.asarray((a - b) ** 2, dtype=np.float32)
